# revision 39
# baseline (speedup 1.0000x reference)
"""Trainium2 Bass kernel for nn_Block_76519137345684 (Spikformer-style block:
spiking self-attention + spiking gated MLP with training-mode BatchNorm).

Strategy
- Data-parallel over batch B across 8 NeuronCores (16 batch each). BN batch
  statistics (per-channel sum / sum-of-squares) are AllReduced across cores.
- Activations live channel-on-partition ("transposed"): (C, rows) with rows
  r = ((t*16 + b)*64 + n); LIF timesteps are contiguous 1024-column slices.
- BN+LIF are folded: with a = g*rsqrt(var+eps) > 0 and c = be - mu*a, the
  LIF recurrence is computed on E_t = 2*mem_t/a - c/a:
      E_t = 0.5*(E_{t-1} + c')*(1 - s_{t-1}) + z_t,  s_t = H(E_t - phi),
  with per-channel c' = c/a and phi = 2*thr/a - c' applied as per-partition
  scalars; z = W@x (bias cancels out of BN entirely).
- Attention uses associativity: y = q @ (k^T v) * scale. All spike operands
  are exact in fp16; k^T v is computed per (t,b) with k,v transposed to
  row-major via PE transposes; per-head block structure is enforced with a
  0.125-scaled block-diagonal mask.
- All matmuls run in fp16 (fp16 x fp16 products are exact in the f32 PSUM
  accumulator, so this matches f32r numerics on fp16-quantized data).
- Host-interface traffic is minimized (the axon execute path re-ships input
  bytes every run at ~12-15 GB/s aggregate): x ships as fp16, the weights
  ship once as a 1/8 shard per core (fp16 blob) and are AllGathered on-chip,
  the attention mask is built on-chip with memsets, and the BN (g, be)
  vectors ship as one packed array.
- Cross-phase intermediates (y-spikes, fc1 z) stay SBUF-resident in
  overlapping tile pools (LIFO close order); only q/k/v spikes, xmid and
  the gated product round-trip through DRAM.
- Depthwise 3x3 conv: spikes are written into a zero-padded (12x10) plane
  layout; 9 shifted multiply-accumulate taps via scalar_tensor_tensor on
  vector/gpsimd engines (fp16, alignment kept even via a +1-shifted copy).
"""
import sys
sys.path.insert(0, '/opt/trn_rl_repo')
import numpy as np

import concourse.bass as bass
import concourse.mybir as mybir
import concourse.tile as tile
from concourse.tile import add_dep_helper

T, B, N, C = 4, 128, 64, 384
HID, CH, HEADS, HD = 1536, 768, 12, 32
NCORES = 8
BS = B // NCORES
R = T * BS * N              # 4096 rows per core
TC = BS * N                 # 1024 cols per timestep
COUNT = T * B * N           # 32768 rows globally (BN stat count)
EPS = 1e-5
PADW = 12
PADP = PADW * 10            # 120 per frame plane
NFR = T * BS                # 64 frames
PADL = NFR * PADP + 32

F32 = mybir.dt.float32
F32R = mybir.dt.float32r
F16 = mybir.dt.float16
ALU = mybir.AluOpType
ACTF = mybir.ActivationFunctionType

_ctr = [0]


def _fix_multiwaits(nc):
    """walrus here accepts max 1 sync-wait per instruction: split extras
    onto same-engine NOPs."""
    for f in nc.m.functions:
        for bb in f.blocks:
            new_insts = []
            for inst in bb.instructions:
                si = inst.sync_info
                ow = list(si.on_wait) if (si and si.on_wait) else []
                if len(ow) > 1:
                    for w in ow[:-1]:
                        _ctr[0] += 1
                        new_insts.append(mybir.InstNoOp(
                            name=f"I-waitnop-{_ctr[0]}", engine=inst.engine,
                            sync_info=mybir.SyncInfo(on_wait=[w], on_update=[]),
                            bass_nofuse=True))
                    si.on_wait = [ow[-1]]
                new_insts.append(inst)
            bb.instructions[:] = new_insts


# packed fp16 weight blob: rows of 1024 elems; per-name ptile blocks are
# [128, co] flattened partition-major.  Row offsets:
WOFF = {"q": 0, "k": 144, "v": 288, "p": 432, "fc1": 576, "fc2": 1152}
WROWS = 1440
WSHARD = WROWS // NCORES
# pv (g, be) stacked rows in pvall
PVOFF = {"q": 0, "k": 384, "v": 768, "p": 1152, "fc1": 1536, "dw": 3072,
         "fc2": 3840}


def build_kernel(debug_taps=False, timing=False, stop_after=None):
    nc = bass.Bass("TRN2", target_bir_lowering=False, debug=False,
                   num_devices=NCORES)

    xT_in = nc.declare_dram_parameter("xT", [C, R], F16, isOutput=False)
    wsh_in = nc.declare_dram_parameter("wsh", [WSHARD, 1024], F16,
                                       isOutput=False)
    pv_in = nc.declare_dram_parameter("pvall", [4224, 2], F32, isOutput=False)
    ident_in = nc.declare_dram_parameter("ident", [128, 128], F16, isOutput=False)
    kv_in = nc.declare_dram_parameter("convk", [CH, 9], F32, isOutput=False)
    if timing:
        out_d = nc.dram_tensor("out", [C, R], F32)
        tok_d = nc.declare_dram_parameter("tok", [128, 1], F32, isOutput=True)
    else:
        out_d = nc.declare_dram_parameter("out", [C, R], F32, isOutput=True)
        tok_d = None

    dbg = {}
    if debug_taps:
        dbg_rm = True
        for nm, npt, dt in [("z_q", 3, F32), ("s_q", 3, F16), ("s_k", 3, F16),
                            ("s_v", 3, F16), ("z_y", 3, F16), ("s_y", 3, F16),
                            ("z_p", 3, F32), ("xmid", 3, F32),
                            ("z_fc1", 12, F32), ("z_conv", 6, F16),
                            ("s_conv", 6, F16), ("gated", 6, F16),
                            ("z_fc2", 3, F32)]:
            dbg[nm] = nc.declare_dram_parameter(f"dbg_{nm}", [npt * 128, R],
                                                dt, isOutput=True)
        for nm in ("rm_k", "rm_v"):
            dbg[nm] = nc.declare_dram_parameter(f"dbg_{nm}", [128, 6 * R],
                                                F16, isOutput=True)

    cc = {}
    for name, co in [("q", C), ("k", C), ("v", C), ("p", C),
                     ("fc1", HID), ("dw", CH), ("fc2", C)]:
        cci = nc.dram_tensor(f"cci_{name}", [co, 2], F32)
        cco = nc.dram_tensor(f"cco_{name}", [co, 2], F32, addr_space="Shared")
        cc[name] = (cci, cco)

    wsh_d = nc.dram_tensor("wsh_d", [WSHARD, 1024], F16)
    wfull = nc.dram_tensor("wfull", [WROWS, 1024], F16, addr_space="Shared")

    xmid_sp = nc.dram_tensor("xmid_spill", [C, R], F16)
    zfc1_sp = nc.dram_tensor("zfc1_spill", [HID, R], F16)

    with tile.TileContext(nc, pool_alloc_mode="queue") as tc:
        _body(nc, tc, xT_in, (wsh_in, wsh_d, wfull), pv_in, ident_in, kv_in,
              out_d, tok_d, cc, xmid_sp, zfc1_sp, dbg, stop_after)
    _fix_multiwaits(nc)
    return nc


def _body(nc, tc, xT_in, w3, pv_in, ident_in, kv_in, out_d, tok_d,
          cc, xmid_sp, zfc1_sp, dbg, stop_after=None):
    from contextlib import ExitStack
    wsh_in, wsh_d, wfull = w3

    # spike DRAM buffers (cross-phase hand-off)
    s_d = {name: nc.dram_tensor(f"s{name}_d", [C, R], F16)
           for name in ("q", "k", "v", "y")}
    gated_d = nc.dram_tensor("gated_d", [CH, R], F16)

    # ---------- long-lived small pools ----------
    ctxL = ExitStack()
    const_p = ctxL.enter_context(tc.tile_pool(name="const", bufs=1))
    stat_p = ctxL.enter_context(tc.tile_pool(name="stats", bufs=1))
    scr_p = ctxL.enter_context(tc.tile_pool(name="scr", bufs=2))
    lif_p = ctxL.enter_context(tc.tile_pool(name="lifp", bufs=1))
    ps_mm = ctxL.enter_context(tc.tile_pool(name="psmm", bufs=4, space="PSUM"))
    ps_at = ctxL.enter_context(tc.tile_pool(name="psat", bufs=1, space="PSUM"))

    # weight-shard AllGather: param -> internal dram -> AllGather -> wfull
    wdma = nc.sync.dma_start(wsh_d[:], wsh_in[:])
    ag = nc.gpsimd.collective_compute(
        "AllGather", mybir.AluOpType.bypass,
        replica_groups=[list(range(NCORES))],
        ins=[wsh_d[:]], outs=[wfull[:]])
    add_dep_helper(ag.ins, wdma.ins, reason="ag waits wsh dma")

    def load_wtile(pool, name, i, co):
        """Load ptile i of weight `name` ([128, co] fp16) from wfull."""
        nrows = co // 8
        r0 = WOFF[name] + i * nrows
        src = wfull[r0:r0 + nrows, :].rearrange("r c -> (r c)") \
            .rearrange("(p k) -> p k", k=co)
        wt16 = pool.tile([128, co], F16, tag=f"w16_{name}{i}",
                         name=f"w16_{name}{i}")
        d = nc.sync.dma_start(wt16[:], src)
        add_dep_helper(d.ins, ag.ins, reason="wtile waits allgather")
        return wt16

    ident = const_p.tile([128, 128], F16, tag="ident", name="ident")
    nc.sync.dma_start(ident[:], ident_in[:])
    mask = const_p.tile([128, 512], F16, tag="mask", name="mask")
    nc.vector.memset(mask[:], 0.0)
    for blk in range(4):
        for h in range(4):
            nc.vector.memset(
                mask[h * 32:(h + 1) * 32,
                     blk * 128 + h * 32:blk * 128 + (h + 1) * 32], 0.125)
    convk = const_p.tile([128, 9 * 6], F32, tag="convk", name="convk")
    for i in range(6):
        nc.sync.dma_start(convk[:, 9 * i:9 * (i + 1)],
                          kv_in[128 * i:128 * (i + 1), :])
    pvec = {}
    for name, npt in [("q", 3), ("k", 3), ("v", 3), ("p", 3),
                      ("fc1", 12), ("dw", 6), ("fc2", 3)]:
        pv = const_p.tile([128, 2 * npt], F32, tag=f"pv_{name}",
                          name=f"pv_{name}")
        for i in range(npt):
            nc.sync.dma_start(pv[:, 2 * i:2 * i + 2],
                              pv_in[PVOFF[name] + 128 * i:
                                    PVOFF[name] + 128 * (i + 1), :])
        pvec[name] = pv

    STT = {}
    for name, npt in [("q", 3), ("k", 3), ("v", 3), ("p", 3),
                      ("fc1", 12), ("dw", 6), ("fc2", 3)]:
        STT[name] = (
            stat_p.tile([128, 8 * npt], F32, tag=f"sum_{name}", name=f"sum_{name}"),
            stat_p.tile([128, 8 * npt], F32, tag=f"sq_{name}", name=f"sq_{name}"),
            stat_p.tile([128, 2 * npt], F32, tag=f"st_{name}", name=f"st_{name}"),
            stat_p.tile([128, 2 * npt], F32, tag=f"cp_{name}", name=f"cp_{name}"),
        )

    # ---------- helpers ----------
    def emit_linear(name, wt, rhs, n_co, n_ci, z_alloc, z_done):
        sump, sqp = STT[name][0], STT[name][1]
        for co in range(n_co):
            z = z_alloc(co)
            for cg in range(2):
                pss = [ps_mm.tile([128, 512], F32, tag="ps", name=f"ps{name}{co}{cg}{j}")
                       for j in range(4)]
                for ci in range(n_ci):
                    for ch in range(4):
                        nc.tensor.matmul(
                            pss[ch][:],
                            lhsT=wt[ci][:, co * 128:(co + 1) * 128],
                            rhs=rhs[ci][:, (cg * 4 + ch) * 512:(cg * 4 + ch + 1) * 512],
                            start=(ci == 0), stop=(ci == n_ci - 1))
                for ch in range(4):
                    g = cg * 4 + ch
                    sl = slice(g * 512, (g + 1) * 512)
                    nc.scalar.activation(z[:, sl], pss[ch][:], ACTF.Copy,
                                         accum_out=sump[:, co * 8 + g:co * 8 + g + 1])
                    sq = scr_p.tile([128, 512], F16, tag="sqscr", name="sqscr")
                    if g % 2 == 0:
                        nc.vector.scalar_tensor_tensor(
                            sq[:], z[:, sl], 1.0, z[:, sl], ALU.mult, ALU.mult,
                            accum_out=sqp[:, co * 8 + g:co * 8 + g + 1])
                    else:
                        nc.scalar.activation(
                            sq[:], pss[ch][:], ACTF.Square,
                            accum_out=sqp[:, co * 8 + g:co * 8 + g + 1])
            z_done(co, z)

    def emit_ar(name, npt, ngrp=8):
        sump, sqp, stfin, _ = STT[name]
        for co in range(npt):
            nc.vector.tensor_reduce(stfin[:, 2 * co:2 * co + 1],
                                    sump[:, co * ngrp:(co + 1) * ngrp],
                                    axis=mybir.AxisListType.X, op=ALU.add)
            nc.vector.tensor_reduce(stfin[:, 2 * co + 1:2 * co + 2],
                                    sqp[:, co * ngrp:(co + 1) * ngrp],
                                    axis=mybir.AxisListType.X, op=ALU.add)
        cci, cco = cc[name]
        dmas = []
        for co in range(npt):
            d = nc.sync.dma_start(cci[128 * co:128 * (co + 1), :],
                                  stfin[:, 2 * co:2 * co + 2])
            dmas.append(d)
        ar = nc.gpsimd.collective_compute(
            "AllReduce", ALU.add, replica_groups=[list(range(NCORES))],
            ins=[cci[:]], outs=[cco[:]])
        for d in dmas:
            add_dep_helper(ar.ins, d.ins, reason="ar waits dma_in")
        for co in range(npt):
            d = nc.sync.dma_start(stfin[:, 2 * co:2 * co + 2],
                                  cco[128 * co:128 * (co + 1), :])
            add_dep_helper(d.ins, ar.ins, reason="readback waits ar")

    def emit_params(name, npt, thr=1.0):
        _, _, stfin, cpphi = STT[name]
        pv = pvec[name]
        for i in range(npt):
            s_ = stfin[:, 2 * i:2 * i + 1]
            q_ = stfin[:, 2 * i + 1:2 * i + 2]
            g_ = pv[:, 2 * i:2 * i + 1]
            be_ = pv[:, 2 * i + 1:2 * i + 2]
            cp_ = cpphi[:, 2 * i:2 * i + 1]
            phi_ = cpphi[:, 2 * i + 1:2 * i + 2]
            w = scr_p.tile([128, 4], F32, tag="pscr", name="pscr")
            mean, var, sd, gi = (w[:, j:j + 1] for j in range(4))
            nc.vector.tensor_scalar(mean, s_, 1.0 / COUNT, None, ALU.mult)
            nc.vector.tensor_scalar(var, q_, 1.0 / COUNT, None, ALU.mult)
            nc.vector.scalar_tensor_tensor(var, mean, mean[:, 0:1], var,
                                           ALU.mult, ALU.subtract)
            nc.vector.tensor_scalar(var, var, -1.0, EPS, ALU.mult, ALU.add)
            nc.scalar.sqrt(sd, var)
            nc.vector.reciprocal(gi, g_)
            nc.vector.tensor_scalar(cp_, be_, sd[:, 0:1], None, ALU.mult)
            nc.vector.tensor_scalar(cp_, cp_, gi[:, 0:1], None, ALU.mult)
            nc.vector.tensor_tensor(cp_, cp_, mean, ALU.subtract)
            nc.vector.tensor_scalar(phi_, sd, gi[:, 0:1], None, ALU.mult)
            nc.vector.tensor_scalar(phi_, phi_, 2.0 * thr, cp_[:, 0:1],
                                    ALU.mult, ALU.subtract)

    def emit_lif(name, z, out_ap_fn, plain=False, pt_off=0, edt=F32,
                 spike_writer=None):
        """LIF over one ptile's z (128, R)."""
        cpphi = None if plain else STT[name][3]
        E = lif_p.tile([128, TC], edt, tag=f"lifE{edt}", name="lifE", bufs=1)
        r_ = lif_p.tile([128, TC], edt, tag=f"lifr{edt}", name="lifr", bufs=1)
        sc = lif_p.tile([128, TC], F16, tag="lifsc", name="lifsc", bufs=1)
        if plain:
            cp_s, phi_s = 0.0, 1.0
        else:
            cp_s = cpphi[:, 2 * pt_off:2 * pt_off + 1]
            phi_s = cpphi[:, 2 * pt_off + 1:2 * pt_off + 2]
        for t in range(T):
            Ecur = z[:, 0:TC] if t == 0 else E[:]
            if spike_writer is not None:
                spike_writer(t, Ecur, phi_s)
            else:
                nc.gpsimd.tensor_scalar(out_ap_fn(t), Ecur, phi_s, None,
                                        ALU.is_ge)
            if t < T - 1:
                nc.gpsimd.tensor_scalar(sc[:], Ecur, phi_s, None, ALU.is_lt)
                if plain:
                    nc.gpsimd.tensor_tensor(r_[:], Ecur, sc[:], ALU.mult)
                else:
                    nc.vector.scalar_tensor_tensor(r_[:], Ecur, cp_s, sc[:],
                                                   ALU.add, ALU.mult)
                nc.vector.scalar_tensor_tensor(
                    E[:], r_[:], 0.5, z[:, (t + 1) * TC:(t + 2) * TC],
                    ALU.mult, ALU.add)

    def dump_rows(nm, row0, t_):
        if nm in dbg:
            nc.sync.dma_start(dbg[nm][row0:row0 + 128, :], t_[:])

    # ============ PHASE 1: q,k,v matmul + AR + LIF -> spikes to DRAM ======
    ctxSPK = ExitStack()
    pSPK = ctxSPK.enter_context(tc.tile_pool(name="pSPK", bufs=1))
    spk = {}
    ctxA = ExitStack()
    ctxA.enter_context(nc.named_scope("ph1_qkv"))
    pA = ctxA.enter_context(tc.tile_pool(name="pA", bufs=1))
    xT = []
    for i in range(3):
        x16 = pA.tile([128, R], F16, tag=f"xT16_{i}", name=f"xT16_{i}")
        nc.sync.dma_start(x16[:], xT_in[128 * i:128 * (i + 1), :])
        xT.append(x16)
    wts = {}
    for name in ("q", "k", "v"):
        wts[name] = [load_wtile(pA, name, i, C) for i in range(3)]

    zs = {}
    for name in ("q", "k", "v"):
        zt = []

        def zalloc(co, name=name, zt=zt):
            z = pA.tile([128, R], F32, tag=f"z{co}", name=f"z{name}{co}", bufs=1)
            zt.append(z)
            return z

        emit_linear(name, wts[name], xT, 3, 3, zalloc, lambda co, z: None)
        zs[name] = zt
        emit_ar(name, 3)
        emit_params(name, 3)
    for pt in range(3):
        dump_rows("z_q", 128 * pt, zs["q"][pt])

    for name in ("q", "k", "v"):
        spk[name] = []
        for pt in range(3):
            st = pSPK.tile([128, R], F16, tag=f"s_{name}{pt}",
                           name=f"s{name}{pt}")
            emit_lif(name, zs[name][pt],
                     lambda t, st=st: st[:, t * TC:(t + 1) * TC], pt_off=pt)
            dump_rows(f"s_{name}", 128 * pt, st)
            spk[name].append(st)
    ctxA.close()
    if stop_after == 'qkv':
        ctxSPK.close(); ctxL.close(); return

    # ============ PHASE 2: transposes + attention + y-LIF ============
    syr = []
    ctxB = ExitStack()
    ctxB.enter_context(nc.named_scope("ph2_attn"))
    pB = ctxB.enter_context(tc.tile_pool(name="pB", bufs=1))
    # rm layout (per pt): per tb a (128, 128) block at col tb*128; rows
    # 0..63 = transposed spikes (n-major), rows 64..127 stay ZERO so mm1
    # can contract over the full K=128 (K=64 matmuls hang on this HW).
    for pt in range(3):
        rmk = pB.tile([128, 64 * 128], F16, tag="rmk", name=f"rmk{pt}",
                      bufs=2)
        rmv = pB.tile([128, 64 * 128], F16, tag="rmv", name=f"rmv{pt}",
                      bufs=2)
        for rmt, name in ((rmk, "k"), (rmv, "v")):
            nc.gpsimd.memset(rmt[64:128, :], 0.0)
            srt = spk[name][pt]
            for grp in range(16):
                pst = ps_at.tile([128, 512], F16, tag="pstr", name="pstr")
                for j in range(4):
                    tb = grp * 4 + j
                    nc.tensor.transpose(pst[0:64, 128 * j:128 * (j + 1)],
                                        srt[:, 64 * tb:64 * (tb + 1)],
                                        ident[:])
                nc.scalar.copy(
                    rmt[0:64, (grp * 4) * 128:(grp * 4 + 4) * 128],
                    pst[0:64, :])
        sqr = spk["q"][pt]
        zy = pB.tile([128, R], F16, tag="zy", name=f"zy{pt}")
        for g4 in range(16):
            mm1ps = ps_at.tile([128, 512], F32, tag="mm1", name="mm1")
            for j in range(4):
                tb = g4 * 4 + j
                base = tb * 128
                nc.tensor.matmul(mm1ps[:, 128 * j:128 * (j + 1)],
                                 lhsT=rmk[:, base:base + 128],
                                 rhs=rmv[:, base:base + 128],
                                 start=True, stop=True)
            m4 = scr_p.tile([128, 512], F16, tag="m4", name="m4")
            nc.vector.tensor_tensor(m4[:], mm1ps[:], mask[:], ALU.mult)
            yps = ps_at.tile([128, 256], F32, tag="yps", name="yps", bufs=2)
            for j in range(4):
                tb = g4 * 4 + j
                nc.tensor.matmul(yps[:, 64 * j:64 * (j + 1)],
                                 lhsT=m4[:, 128 * j:128 * (j + 1)],
                                 rhs=sqr[:, 64 * tb:64 * (tb + 1)],
                                 start=True, stop=True)
            nc.scalar.copy(zy[:, 256 * g4:256 * (g4 + 1)], yps[:])
        dump_rows("z_y", 128 * pt, zy)
        syt = pSPK.tile([128, R], F16, tag=f"sy{pt}", name=f"sy{pt}")
        emit_lif("y", zy, lambda t, syt=syt: syt[:, t * TC:(t + 1) * TC],
                 plain=True, edt=F16)
        dump_rows("s_y", 128 * pt, syt)
        syr.append(syt)
    ctxB.close()
    if stop_after == 'attn':
        ctxSPK.close(); ctxL.close(); return

    # ============ PHASE 3: p-linear + xmid ============
    ctxC = ExitStack()
    ctxC.enter_context(nc.named_scope("ph3_p"))
    pC = ctxC.enter_context(tc.tile_pool(name="pC", bufs=1))
    wt_p = [load_wtile(pC, "p", i, C) for i in range(3)]
    zp = []

    def zalloc_p(co):
        z = pC.tile([128, R], F32, tag=f"zp{co}", name=f"zp{co}")
        zp.append(z)
        return z

    emit_linear("p", wt_p, syr, 3, 3, zalloc_p, lambda co, z: None)
    emit_ar("p", 3)
    emit_params("p", 3)
    for pt in range(3):
        dump_rows("z_p", 128 * pt, zp[pt])
        spt = pC.tile([128, R], F16, tag="sptr", name=f"sp{pt}", bufs=1)
        emit_lif("p", zp[pt], lambda t, spt=spt: spt[:, t * TC:(t + 1) * TC],
                 pt_off=pt)
        xr = pC.tile([128, R], F16, tag="xm", name=f"xm{pt}", bufs=1)
        nc.sync.dma_start(xr[:], xT_in[128 * pt:128 * (pt + 1), :])
        nc.gpsimd.tensor_tensor(xr[:], xr[:], spt[:], ALU.add)
        nc.sync.dma_start(xmid_sp[128 * pt:128 * (pt + 1), :], xr[:])
        dump_rows("xmid", 128 * pt, xr)
    ctxC.close()
    ctxSPK.close()
    if stop_after == 'p':
        ctxL.close(); return

    # ============ PHASE 4: fc1 -> z to DRAM ============
    ctxZB = ExitStack()
    pZB = ctxZB.enter_context(tc.tile_pool(name="pZB", bufs=1))
    ctxZA = ExitStack()
    pZA = ctxZA.enter_context(tc.tile_pool(name="pZA", bufs=1))
    ctxD = ExitStack()
    ctxD.enter_context(nc.named_scope("ph4_fc1"))
    pD = ctxD.enter_context(tc.tile_pool(name="pD", bufs=1))
    xmid = []
    for i in range(3):
        x = pD.tile([128, R], F16, tag=f"xmid{i}", name=f"xmid{i}")
        nc.sync.dma_start(x[:], xmid_sp[128 * i:128 * (i + 1), :])
        xmid.append(x)
    wt_fc1 = [load_wtile(pD, "fc1", i, HID) for i in range(3)]

    zs_fc1 = [None] * 12

    def zalloc_f(co):
        pool = pZA if co < 6 else pZB
        z = pool.tile([128, R], F16, tag=f"zk{co}", name=f"zf{co}")
        zs_fc1[co] = z
        return z

    def zdone_f(co, z):
        if "z_fc1" in dbg:
            nc.sync.dma_start(dbg["z_fc1"][128 * co:128 * (co + 1), :], z[:])

    emit_linear("fc1", wt_fc1, xmid, 12, 3, zalloc_f, zdone_f)
    emit_ar("fc1", 12)
    emit_params("fc1", 12)
    ctxD.close()
    if stop_after == 'fc1':
        ctxZA.close(); ctxZB.close(); ctxL.close(); return

    # ============ PHASE 5a: x1-LIF + conv taps ============
    ctxZ5 = ExitStack()
    ctxZ5.enter_context(nc.named_scope("ph5_conv"))
    z_conv = [pZA.tile([128, R], F16, tag=f"zc{i}", name=f"zc{i}")
              for i in range(6)]
    sump_c, sqp_c, _, _ = STT["dw"]

    ctxE = ExitStack()
    pE = ctxE.enter_context(tc.tile_pool(name="pE", bufs=1))

    def tap_view(src, off):
        return src[:, off:off + NFR * PADP] \
            .rearrange("p (f a) -> p f a", a=PADP)[:, :, 0:8 * PADW] \
            .rearrange("p f (h w) -> p f h w", w=PADW)[:, :, :, 0:8]

    for i in range(6):
        zi = zs_fc1[i]
        xa = pE.tile([128, PADL], F16, tag="cxa", name=f"cxa{i}")
        nc.gpsimd.memset(xa[:], 0.0)

        def x1_writer(t, Ecur, phi_s, xa=xa):
            # per-h 3D writes into the padded plane
            xa3 = xa[:, t * BS * PADP:(t + 1) * BS * PADP] \
                .rearrange("p (f a) -> p f a", f=BS)
            e3 = Ecur[:].rearrange("p (f a) -> p f a", f=BS)
            for h in range(8):
                col = (1 + h) * PADW + 2
                nc.gpsimd.tensor_scalar(xa3[:, :, col:col + 8],
                                        e3[:, :, h * 8:h * 8 + 8],
                                        phi_s, None, ALU.is_ge)

        emit_lif("fc1", zi, None, pt_off=i, spike_writer=x1_writer)
        xb = pE.tile([128, PADL], F16, tag="cxb", name=f"cxb{i}")
        nc.scalar.copy(xb[:, 1:PADL], xa[:, 0:PADL - 1])
        tapi = 0
        for dh in (-1, 0, 1):
            for dw_ in (-1, 0, 1):
                base0 = (1 + dh) * PADW + 2 + dw_
                src, off0 = (xa, base0) if dw_ == 0 else (xb, base0 + 1)
                ks = convk[:, 9 * i + tapi:9 * i + tapi + 1]
                src3 = src[:, 0:NFR * PADP].rearrange("p (f a) -> p f a",
                                                      f=NFR)
                zc3 = z_conv[i][:].rearrange("p (f a) -> p f a", f=NFR)
                for h in range(8):
                    col = off0 + h * PADW
                    in3 = src3[:, :, col:col + 8]
                    out3 = zc3[:, :, h * 8:h * 8 + 8]
                    if tapi == 0:
                        nc.gpsimd.tensor_scalar(out3, in3, ks, None, ALU.mult)
                    elif tapi == 8:
                        nc.vector.scalar_tensor_tensor(
                            out3, in3, ks, out3, ALU.mult, ALU.add,
                            accum_out=sump_c[:, 8 * i + h:8 * i + h + 1])
                    elif i % 2 == 0:
                        nc.vector.scalar_tensor_tensor(
                            out3, in3, ks, out3, ALU.mult, ALU.add)
                    else:
                        tmp = scr_p.tile([128, 512], F16, tag="ctmp",
                                         name=f"ct{i}{tapi}{h}")
                        nc.scalar.activation(tmp[:], in3, ACTF.Copy, scale=ks)
                        tmp3 = tmp[:].rearrange("p (f a) -> p f a", a=8)
                        nc.gpsimd.tensor_tensor(out3, tmp3, out3, ALU.add)
                tapi += 1
        sq = scr_p.tile([128, 512], F16, tag="sqscr", name=f"sqc{i}")
        for g in range(8):
            sl = slice(g * 512, (g + 1) * 512)
            nc.vector.scalar_tensor_tensor(
                sq[:], z_conv[i][:, sl], 1.0, z_conv[i][:, sl],
                ALU.mult, ALU.mult,
                accum_out=sqp_c[:, 8 * i + g:8 * i + g + 1])
    ctxE.close()
    emit_ar("dw", 6)
    emit_params("dw", 6)
    for i in range(6):
        dump_rows("z_conv", 128 * i, z_conv[i])

    # ============ PHASE 5b: x2-LIF + conv-LIF + gating ============
    ctxF = ExitStack()
    pF = ctxF.enter_context(tc.tile_pool(name="pF", bufs=1))
    for i in range(6):
        zi = zs_fc1[6 + i]
        sx2 = pF.tile([128, R], F16, tag=f"gt{i % 2}", name=f"sx2_{i}")
        emit_lif("fc1", zi, lambda t, sx2=sx2: sx2[:, t * TC:(t + 1) * TC],
                 pt_off=6 + i)
        scv = pF.tile([128, R], F16, tag="scv", name=f"scv{i}")
        emit_lif("dw", z_conv[i],
                 lambda t, scv=scv: scv[:, t * TC:(t + 1) * TC],
                 pt_off=i, edt=F16)
        dump_rows("s_conv", 128 * i, scv)
        nc.gpsimd.tensor_tensor(sx2[:], scv[:], sx2[:], ALU.mult)
        nc.sync.dma_start(gated_d[128 * i:128 * (i + 1), :], sx2[:])
        dump_rows("gated", 128 * i, sx2)
    ctxF.close()
    ctxZA.close()
    ctxZB.close()
    ctxZ5.close()
    if stop_after == 'conv':
        ctxL.close(); return

    # ============ PHASE 6: fc2 + final residual ============
    ctxG = ExitStack()
    ctxG.enter_context(nc.named_scope("ph6_fc2"))
    pG = ctxG.enter_context(tc.tile_pool(name="pG", bufs=1))
    gtr = []
    for i in range(6):
        g = pG.tile([128, R], F16, tag=f"gtr{i}", name=f"gtr{i}")
        nc.sync.dma_start(g[:], gated_d[128 * i:128 * (i + 1), :])
        gtr.append(g)
    wt_fc2 = [load_wtile(pG, "fc2", i, C) for i in range(6)]
    zf2 = []

    def zalloc_g(co):
        z = pG.tile([128, R], F16, tag=f"zf2{co}", name=f"zf2{co}")
        zf2.append(z)
        return z

    emit_linear("fc2", wt_fc2, gtr, 3, 6, zalloc_g, lambda co, z: None)
    emit_ar("fc2", 3)
    emit_params("fc2", 3)
    for pt in range(3):
        dump_rows("z_fc2", 128 * pt, zf2[pt])
        so = pG.tile([128, R], F16, tag="so", name=f"so{pt}", bufs=1)
        emit_lif("fc2", zf2[pt], lambda t, so=so: so[:, t * TC:(t + 1) * TC],
                 pt_off=pt)
        xm = pG.tile([128, R], F16, tag="xmr", name=f"xmr{pt}")
        nc.sync.dma_start(xm[:], xmid_sp[128 * pt:128 * (pt + 1), :])
        xo = pG.tile([128, R], F32, tag="xo", name=f"xo{pt}")
        nc.gpsimd.tensor_tensor(xo[:], xm[:], so[:], ALU.add)
        nc.sync.dma_start(out_d[128 * pt:128 * (pt + 1), :], xo[:])
    if tok_d is not None:
        tk = pG.tile([128, 1], F32, tag="tok", name="tk")
        nc.vector.memset(tk[:], 1.0)
        nc.sync.dma_start(tok_d[:], tk[:])
    ctxG.close()
    ctxL.close()


# ---------------- host glue ----------------

def _prep_inputs(inputs):
    x = np.asarray(inputs['x'], np.float32)
    xr = x.reshape(T, B, N, C)
    ident = np.eye(128, dtype=np.float16)

    wblob = np.zeros((WROWS, 1024), np.float16)
    for name in ("q", "k", "v", "p", "fc1", "fc2"):
        wt = np.ascontiguousarray(
            np.asarray(inputs[name + "_w"]).T).astype(np.float16)  # [ci, co]
        ci, co = wt.shape
        nrows = co // 8
        for i in range(ci // 128):
            blk = wt[128 * i:128 * (i + 1)].reshape(nrows, 1024)
            wblob[WOFF[name] + i * nrows:WOFF[name] + (i + 1) * nrows] = blk

    pvall = np.zeros((4224, 2), np.float32)
    for name in ("q", "k", "v", "p", "fc1", "fc2", "dw"):
        g = np.asarray(inputs[name + "_g"], np.float32)
        be = np.asarray(inputs[name + "_be"], np.float32)
        pvall[PVOFF[name]:PVOFF[name] + g.shape[0]] = np.stack([g, be], 1)

    common = {"ident": ident, "pvall": pvall,
              "convk": np.ascontiguousarray(
                  np.asarray(inputs["dw_k"], np.float32).reshape(CH, 9))}

    maps = []
    for c in range(NCORES):
        shard = xr[:, c * BS:(c + 1) * BS]
        xt = np.ascontiguousarray(shard.reshape(R, C).T).astype(np.float16)
        m = dict(common)
        m["xT"] = xt
        m["wsh"] = np.ascontiguousarray(
            wblob[c * WSHARD:(c + 1) * WSHARD])
        maps.append(m)
    return maps


_CACHE = {}


def _get_runner(debug_taps=False, timing=False, stop_after=None):
    key = (debug_taps, timing, stop_after)
    if key not in _CACHE:
        from runner_embed import SpmdRunner
        nc = build_kernel(debug_taps, timing, stop_after)
        _CACHE[key] = SpmdRunner(nc, NCORES)
    return _CACHE[key]


def kernel(**inputs):
    r = _get_runner()
    maps = _prep_inputs(inputs)
    args = r.prep(maps)
    outs = r.run(args)
    res = r.results(outs)
    full = np.empty((T, B, N, C), np.float32)
    for c in range(NCORES):
        o = res[c]["out"]
        full[:, c * BS:(c + 1) * BS] = o.T.reshape(T, BS, N, C)
    return np.ascontiguousarray(full.reshape(T * B, N, C))


# ---- embedded SPMD runner module ----
import types
runner_embed = types.ModuleType("runner_embed")
sys.modules["runner_embed"] = runner_embed
exec(r'''
import sys
sys.path.insert(0, '/opt/trn_rl_repo')
import numpy as np
import jax
from jax.sharding import Mesh, PartitionSpec
from jax.experimental.shard_map import shard_map
import concourse.bass as bass
import concourse.mybir as mybir
from concourse.bass2jax import _bass_exec_p, install_neuronx_cc_hook, partition_id_tensor


class SpmdRunner:
    def __init__(self, nc, n_cores):
        install_neuronx_cc_hook()
        self.nc = nc
        self.n_cores = n_cores
        partition_name = nc.partition_id_tensor.name if nc.partition_id_tensor else None
        in_names, out_names, out_avals, zero_outs = [], [], [], []
        for alloc in nc.m.functions[0].allocations:
            if not isinstance(alloc, mybir.MemoryLocationSet):
                continue
            name = alloc.memorylocations[0].name
            if alloc.kind == "ExternalInput":
                if name != partition_name:
                    in_names.append(name)
            elif alloc.kind == "ExternalOutput":
                shape = tuple(alloc.tensor_shape)
                dtype = mybir.dt.np(alloc.dtype)
                out_names.append(name)
                out_avals.append(jax.core.ShapedArray(shape, dtype))
                zero_outs.append(np.zeros(shape, dtype))
        self.in_names, self.out_names = in_names, out_names
        self.out_avals, self.zero_outs = out_avals, zero_outs
        n_params = len(in_names)
        n_outs = len(out_avals)
        all_in_names = list(in_names) + list(out_names)
        if partition_name is not None:
            all_in_names.append(partition_name)
        self.n_params = n_params

        def _body(*args):
            operands = list(args)
            if partition_name is not None:
                operands.append(partition_id_tensor())
            outs = _bass_exec_p.bind(
                *operands, out_avals=tuple(out_avals),
                in_names=tuple(all_in_names), out_names=tuple(out_names),
                lowering_input_output_aliases=(),
                sim_require_finite=True, sim_require_nnan=True, nc=nc)
            return tuple(outs)

        devices = jax.devices()[:n_cores]
        assert len(devices) == n_cores
        mesh = Mesh(np.asarray(devices), ("core",))
        in_specs = (PartitionSpec("core"),) * (n_params + n_outs)
        out_specs = (PartitionSpec("core"),) * n_outs
        self.fn = jax.jit(
            shard_map(_body, mesh=mesh, in_specs=in_specs,
                      out_specs=out_specs, check_rep=False),
            keep_unused=True)

    def prep(self, in_maps):
        per_core = [[np.asarray(m[name]) for name in self.in_names]
                    for m in in_maps]
        concat_in = [np.concatenate([per_core[c][i] for c in range(self.n_cores)], axis=0)
                     for i in range(self.n_params)]
        concat_zeros = [np.zeros((self.n_cores * z.shape[0], *z.shape[1:]), z.dtype)
                        for z in self.zero_outs]
        return [jax.device_put(a) for a in concat_in + concat_zeros]

    def run(self, args):
        outs = self.fn(*args)
        jax.block_until_ready(outs)
        return outs

    def results(self, outs):
        res = []
        for c in range(self.n_cores):
            res.append({name: np.asarray(outs[i]).reshape(self.n_cores, *self.out_avals[i].shape)[c]
                        for i, name in enumerate(self.out_names)})
        return res

    def time_it(self, args, iters=20, warmup=3):
        import time
        for _ in range(warmup):
            self.run(args)
        ts = []
        for _ in range(iters):
            t0 = time.perf_counter()
            self.run(args)
            ts.append(time.perf_counter() - t0)
        ts = np.array(ts)
        return dict(min=ts.min(), median=float(np.median(ts)), mean=ts.mean())
''', runner_embed.__dict__)

